# revision 1
# baseline (speedup 1.0000x reference)
"""Trainium2 Bass kernel for nn_BetaVAEMark7Decoder.

Strategy: all six layers are (blocks of) matmuls on the TensorEngine in
float32r. The up-conv/tconv pairs are fused on the host into composite
banded weight blocks (up1*tc1, up2*tc2, up3*tc3), so the device work is
pure data-parallel over batch: 4096 rows split 512 per NeuronCore.

Layouts: inner layers keep activations as [feature_partitions, (h, b)]
with the matmul moving dim = batch; the final fused layer swaps operands
(activations become the stationary lhsT, weights the moving rhs) so PSUM
comes out batch-major ([128 b, out-features]) and the NCHW output DMA is
contiguous per partition.
"""
import numpy as np
from contextlib import ExitStack

import concourse.bass as bass
import concourse.tile as tile
from concourse import bacc, mybir
from concourse.bass_utils import run_bass_kernel_spmd

F32 = mybir.dt.float32
F32R = mybir.dt.float32r
BF16 = mybir.dt.bfloat16
AF = mybir.ActivationFunctionType
OP = mybir.AluOpType

NCORES = 8
BCORE = 512          # batch rows per core
HALF = 512           # fused2/fused3 batch span (full)
CH = 128             # fused3 output-chunk batch size

X1_WIN = [(0, 3), (1, 4), (3, 4), (5, 3)]
X3_WIN = [(0, 5), (2, 7), (6, 7), (10, 6)]
A3_WIN = [(0, 12, 0, 10), (8, 12, 11, 18), (16, 12, 19, 26), (20, 12, 27, 31)]
A3_OWN = [(0, 0, 12), (1, 12, 20), (2, 20, 28), (3, 28, 32)]
HG = [(0, 2), (2, 2), (4, 1)]


# ---------------- host-side weight factorization ----------------
def _precompute(w):
    P = {}
    w_lin, b_lin = w["w_lin"], w["b_lin"]
    lhs_lin = np.zeros((7, 256), np.float32)
    c_lin = np.zeros(256, np.float32)
    for wi in range(8):
        for ci in range(32):
            lhs_lin[:, wi * 32 + ci] = w_lin[:, ci * 8 + wi]
            c_lin[wi * 32 + ci] = b_lin[ci * 8 + wi]
    P["lhs_lin"], P["c_lin"] = lhs_lin, c_lin

    w_up1, b_up1, w_tc1, b_tc1 = w["w_up1"], w["b_up1"], w["w_tc1"], w["b_tc1"]
    K1 = np.zeros((5, 2, 3, 32, 16), np.float32)
    for hh in range(5):
        for s in range(2):
            for dh in range(3):
                hp = hh + 1 - dh
                if not (0 <= hp < 5):
                    continue
                for dw in range(3):
                    t = s + 1 - dw
                    dj = int(np.floor(t / 2))
                    kw = t - 2 * dj
                    K1[hh, s, dj + 1] += np.einsum("ic,cd->id", w_up1[hp, kw], w_tc1[dh, dw])
    c1 = np.zeros((5, 16, 16), np.float32)
    for hh in range(5):
        for ww in range(16):
            acc = b_tc1.copy()
            for dh in range(3):
                if not (0 <= hh + 1 - dh < 5):
                    continue
                for dw in range(3):
                    if not (0 <= ww + 1 - dw < 16):
                        continue
                    acc = acc + b_up1 @ w_tc1[dh, dw]
            c1[hh, ww] = acc
    P["K1"], P["c1"] = K1, c1

    w_up2, b_up2, w_tc2, b_tc2 = w["w_up2"], w["b_up2"], w["w_tc2"], w["b_tc2"]
    K2 = np.zeros((5, 2, 3, 3, 16, 8), np.float32)
    for r in range(5):
        for s in range(2):
            for dh in range(3):
                u = r + 1 - dh
                di = int(np.floor(u / 5))
                kh = u - 5 * di
                for dw in range(3):
                    t = s + 1 - dw
                    dj = int(np.floor(t / 2))
                    kw = t - 2 * dj
                    K2[r, s, di + 1, dj + 1] += np.einsum("ic,cd->id", w_up2[kh, kw], w_tc2[dh, dw])
    P["K2"] = K2
    P["BB2"] = np.einsum("c,hwcd->hwd", b_up2, w_tc2)
    P["b_tc2"] = b_tc2

    w_up3, b_up3, w_tc3, b_tc3 = w["w_up3"], w["b_up3"], w["w_tc3"], w["b_tc3"]
    K3 = np.zeros((2, 2, 3, 3, 8, 6), np.float32)
    for r in range(2):
        for s in range(2):
            for dh in range(3):
                u = r + 1 - dh
                di = int(np.floor(u / 2))
                kh = u - 2 * di
                for dw in range(3):
                    t = s + 1 - dw
                    dj = int(np.floor(t / 2))
                    kw = t - 2 * dj
                    K3[r, s, di + 1, dj + 1] += np.einsum("ic,cd->id", w_up3[kh, kw], w_tc3[dh, dw])
    P["K3"] = K3
    P["BB3"] = np.einsum("c,hwcd->hwd", b_up3, w_tc3)
    P["b_tc3"] = b_tc3
    return P


def _fused1_blocks(P):
    K1 = P["K1"]
    blocks, biases = {}, {}
    for g, (h0, nh) in enumerate(HG):
        for a in range(4):
            wi0, nwi = X1_WIN[a]
            M = nh * 4 * 16
            B = np.zeros((nwi * 32, M), np.float32)
            bias = np.zeros(M, np.float32)
            for hi in range(nh):
                hh = h0 + hi
                for wl in range(4):
                    ww = 4 * a + wl
                    j, s = ww // 2, ww % 2
                    for c2 in range(16):
                        col = hi * 64 + wl * 16 + c2
                        bias[col] = P["c1"][hh, ww, c2]
                        for wi_l in range(nwi):
                            dj = (wi0 + wi_l) - j
                            if -1 <= dj <= 1:
                                B[wi_l * 32:(wi_l + 1) * 32, col] = K1[hh, s, dj + 1, :, c2]
            blocks[(g, a)] = B
            biases[(g, a)] = bias
    return blocks, biases


def _fused2_blocks(P):
    K2, BB2, b_tc2 = P["K2"], P["BB2"], P["b_tc2"]

    def col_bias(Hh, Ww, c3):
        acc = b_tc2[c3]
        for dh in range(3):
            if not (0 <= Hh + 1 - dh < 25):
                continue
            for dw in range(3):
                if not (0 <= Ww + 1 - dw < 32):
                    continue
                acc += BB2[dh, dw, c3]
        return acc

    blocks, biases = {}, {}
    for a in range(4):
        j0, nj = X3_WIN[a]
        Kr = nj * 16
        B = np.zeros((Kr, 128), np.float32)
        for ri, r in enumerate((1, 2)):
            for wl in range(8):
                Ww = 8 * a + wl
                j, s = Ww // 2, Ww % 2
                for c3 in range(8):
                    col = ri * 64 + wl * 8 + c3
                    for jl in range(nj):
                        dj = (j0 + jl) - j
                        if -1 <= dj <= 1:
                            B[jl * 16:(jl + 1) * 16, col] = K2[r, s, 1, dj + 1, :, c3]
        blocks[("r12", a)] = B
        bias = np.zeros(128, np.float32)
        for ri, r in enumerate((1, 2)):
            for wl in range(8):
                for c3 in range(8):
                    bias[ri * 64 + wl * 8 + c3] = col_bias(5 + r, 8 * a + wl, c3)
        biases[("r12", a)] = bias

        B = np.zeros((Kr, 64), np.float32)
        for wl in range(8):
            Ww = 8 * a + wl
            j, s = Ww // 2, Ww % 2
            for c3 in range(8):
                for jl in range(nj):
                    dj = (j0 + jl) - j
                    if -1 <= dj <= 1:
                        B[jl * 16:(jl + 1) * 16, wl * 8 + c3] = K2[3, s, 1, dj + 1, :, c3]
        blocks[("r3", a)] = B
        bias = np.zeros(64, np.float32)
        for wl in range(8):
            for c3 in range(8):
                bias[wl * 8 + c3] = col_bias(8, 8 * a + wl, c3)
        biases[("r3", a)] = bias

        for kind, r, dis in (("r0", 0, (-1, 0)), ("r4", 4, (0, 1))):
            for di in dis:
                B = np.zeros((Kr, 64), np.float32)
                for wl in range(8):
                    Ww = 8 * a + wl
                    j, s = Ww // 2, Ww % 2
                    for c3 in range(8):
                        for jl in range(nj):
                            dj = (j0 + jl) - j
                            if -1 <= dj <= 1:
                                B[jl * 16:(jl + 1) * 16, wl * 8 + c3] = K2[r, s, di + 1, dj + 1, :, c3]
                blocks[(kind, a, di)] = B
            # bias: interior-H version and edge version (i=0 for r0, i=4 for r4)
            for tag, i in (("mid", 2), ("edge", 0 if kind == "r0" else 4)):
                bias = np.zeros(64, np.float32)
                for wl in range(8):
                    for c3 in range(8):
                        bias[wl * 8 + c3] = col_bias(5 * i + r, 8 * a + wl, c3)
                biases[(kind, a, tag)] = bias
    return blocks, biases


def _fused3_blocks(P):
    """bf16 rhs blocks per (t, di[, iclass]): rows = (jl*8+ci) for jl<12, bias row 96.
    cols (r, s, jc, c4) r-major."""
    K3, BB3, b_tc3 = P["K3"], P["BB3"], P["b_tc3"]
    blocks = {}
    for t, (j0, nj, lo, hi) in enumerate(A3_WIN):
        njc = hi - lo + 1
        N = 4 * njc * 6

        def colidx(r, s, jc, c4):
            return ((r * 6 + c4) * njc + (jc - lo)) * 2 + s

        for di in (-1, 0, 1):
            B = np.zeros((97, N), np.float32)
            for r in range(2):
                for s in range(2):
                    for jc in range(lo, hi + 1):
                        for c4 in range(6):
                            col = colidx(r, s, jc, c4)
                            for jl in range(nj):
                                dj = (j0 + jl) - jc
                                if -1 <= dj <= 1:
                                    B[jl * 8:(jl + 1) * 8, col] = K3[r, s, di + 1, dj + 1, :, c4]
            if di != 0:
                blocks[(t, di)] = B
                continue
            for iclass in range(3):
                Bi = B.copy()
                for r in range(2):
                    for s in range(2):
                        for jc in range(lo, hi + 1):
                            for c4 in range(6):
                                acc = b_tc3[c4]
                                for dh in range(3):
                                    u = r + 1 - dh
                                    di_ = int(np.floor(u / 2))
                                    ok = (iclass == 0) or (iclass == 1 and di_ >= 0) \
                                        or (iclass == 2 and di_ <= 0)
                                    if not ok:
                                        continue
                                    for dw in range(3):
                                        tt = s + 1 - dw
                                        dj_ = int(np.floor(tt / 2))
                                        if 0 <= jc + dj_ < 32:
                                            acc += BB3[dh, dw, c4]
                                Bi[96, colidx(r, s, jc, c4)] = acc
                blocks[(t, 0, iclass)] = Bi
    return blocks


class _Pack:
    """Packs [K, M] blocks into one [128, cols] array; remembers offsets."""

    def __init__(self):
        self.cols = 0
        self.reg = {}
        self.items = []

    def add(self, key, arr):
        K, M = arr.shape
        self.reg[key] = (self.cols, K, M)
        self.items.append(arr)
        self.cols += M

    def build(self):
        out = np.zeros((128, self.cols), np.float32)
        c = 0
        for arr in self.items:
            K, M = arr.shape
            out[:K, c:c + M] = arr
            c += M
        return out


def _make_packs(inputs):
    P = _precompute(inputs)
    f1b, f1bias = _fused1_blocks(P)
    f2b, f2bias = _fused2_blocks(P)
    f3b = _fused3_blocks(P)

    wp = _Pack()
    wp.add("lin0", P["lhs_lin"][:, 0:128])
    wp.add("lin1", P["lhs_lin"][:, 128:256])
    for g in range(3):
        for a in range(4):
            wp.add(("f1", g, a), f1b[(g, a)])
    wb = _Pack()
    for a in range(4):
        wb.add(("f2r12", a), f2b[("r12", a)])
        wb.add(("f2r3", a), f2b[("r3", a)])
        for di in (-1, 0):
            wb.add(("f2r0", a, di), f2b[("r0", a, di)])
        for di in (0, 1):
            wb.add(("f2r4", a, di), f2b[("r4", a, di)])
    for t in range(4):
        for key in [(t, 0, 0), (t, 0, 1), (t, 0, 2), (t, -1), (t, 1)]:
            wb.add(("f3",) + key, f3b[key])

    bp = _Pack()
    bp.add("blin0", P["c_lin"][0:128].reshape(-1, 1))
    bp.add("blin1", P["c_lin"][128:256].reshape(-1, 1))
    for g in range(3):
        for a in range(4):
            bp.add(("b1", g, a), f1bias[(g, a)].reshape(-1, 1))
    for a in range(4):
        bp.add(("b2r12", a), f2bias[("r12", a)].reshape(-1, 1))
        bp.add(("b2r3", a), f2bias[("r3", a)].reshape(-1, 1))
        for tag in ("mid", "edge"):
            bp.add(("b2r0", a, tag), f2bias[("r0", a, tag)].reshape(-1, 1))
            bp.add(("b2r4", a, tag), f2bias[("r4", a, tag)].reshape(-1, 1))
    return wp, bp, wb


# ---------------- device program ----------------
_PROG = {}


def _lim(s):
    if s == 0:
        return 128
    if s == 64:
        return 64
    return 32


def _pieces(p0, d0, n):
    """Split a partition-range copy into HW-legal (offset, count) pieces.
    Both starts must be 32-aligned and each piece must obey the buddy rule."""
    assert p0 % 32 == 0 and d0 % 32 == 0, (p0, d0, n)
    out = []
    off = 0
    while off < n:
        s1, s2 = (p0 + off) % 128, (d0 + off) % 128
        c = min(n - off, _lim(s1), _lim(s2))
        out.append((off, c))
        off += c
    return out


def _build_program(wcols, bcols, wbcols):
    key = (wcols, bcols, wbcols)
    if key in _PROG:
        return _PROG[key]
    nc = bacc.Bacc("TRN2", target_bir_lowering=False, debug=False, num_devices=NCORES)
    lat_ap = nc.dram_tensor("latent", [BCORE, 7], F32, kind="ExternalInput").ap()
    wp_ap = nc.dram_tensor("wpack", [128, wcols], F32, kind="ExternalInput").ap()
    bp_ap = nc.dram_tensor("bpack", [128, bcols], F32, kind="ExternalInput").ap()
    wb_ap = nc.dram_tensor("wbpack", [128, wbcols], BF16, kind="ExternalInput").ap()
    out_ap = nc.dram_tensor("out", [BCORE, 6, 50, 64], F32, kind="ExternalOutput").ap()
    with tile.TileContext(nc) as tc:
        with ExitStack() as ctx:
            _emit(ctx, tc, nc, lat_ap, wp_ap, bp_ap, wb_ap, out_ap,
                  _build_program.wreg, _build_program.breg, _build_program.wbreg)
    nc.compile()
    _PROG[key] = nc
    return nc


def _emit(ctx, tc, nc, lat_ap, wp_ap, bp_ap, wb_ap, out_ap, wreg, breg, wbreg):
    wcols = wp_ap.shape[1]
    bcols = bp_ap.shape[1]
    wbcols = wb_ap.shape[1]

    consts = ctx.enter_context(tc.tile_pool(name="consts", bufs=1))
    bounce = ctx.enter_context(tc.tile_pool(name="bounce", bufs=2))
    x1p = ctx.enter_context(tc.tile_pool(name="x1", bufs=1))
    x3p = ctx.enter_context(tc.tile_pool(name="x3", bufs=1))
    a3p = ctx.enter_context(tc.tile_pool(name="a3", bufs=1))
    stgp = ctx.enter_context(tc.tile_pool(name="stg", bufs=4))
    tmpp = ctx.enter_context(tc.tile_pool(name="tmp", bufs=4))
    ps_ctx = ExitStack()
    psmall = ps_ctx.enter_context(tc.tile_pool(name="psA", bufs=2, space="PSUM"))

    # ---- constants ----
    wp_r = consts.tile([128, wcols], F32R)
    for c0 in range(0, wcols, 512):
        n = min(512, wcols - c0)
        bt = bounce.tile([128, 512], F32, tag="bounce", name=f"bw{c0}")
        nc.sync.dma_start(bt[:, :n], wp_ap[:, c0:c0 + n])
        nc.vector.tensor_copy(wp_r[:, c0:c0 + n], bt[:, :n])
    wbt = consts.tile([128, wbcols], BF16)
    nc.sync.dma_start(wbt[:], wb_ap[:])
    bpt = consts.tile([128, bcols], F32)
    nc.sync.dma_start(bpt[:], bp_ap[:])
    lat_f = consts.tile([7, BCORE], F32)
    nc.sync.dma_start(lat_f[:], lat_ap[:].rearrange("b d -> d b"))
    lat_r = consts.tile([7, BCORE], F32R)
    nc.vector.tensor_copy(lat_r[:], lat_f[:])

    def W(key):
        o, K, M = wreg[key]
        return wp_r[:K, o:o + M]

    def WB(key):
        o, K, M = wbreg[key]
        return wbt[:K, o:o + M]

    def BV(key, p0, n):
        o, K, M = breg[key]
        return bpt[p0:p0 + n, o:o + 1]

    def act_lrelu(dst, src, bias):
        nc.scalar.activation(dst, src, AF.Lrelu, bias=bias, scale=1.0, alpha=0.01)

    def evac(dst_tile, d0, ps, p0, n, bkey, fsl_out, fsl_in=None):
        """lrelu+bias evacuation with partition legality splitting."""
        fsl_in = fsl_in if fsl_in is not None else slice(None)
        for off, cnt in _pieces(p0, d0, n):
            act_lrelu(dst_tile[d0 + off:d0 + off + cnt, fsl_out],
                      ps[p0 + off:p0 + off + cnt, fsl_in],
                      BV(bkey, p0 + off, cnt))

    # ---- lin ----
    psA = psmall.tile([128, BCORE], F32, tag="ps")
    nc.tensor.matmul(psA[:], W("lin0"), lat_r[:], start=True, stop=True)
    psB = psmall.tile([128, BCORE], F32, tag="ps")
    nc.tensor.matmul(psB[:], W("lin1"), lat_r[:], start=True, stop=True)

    x1t = [x1p.tile([X1_WIN[a][1] * 32, BCORE], F32R, tag=f"x1_{a}", name=f"x1_{a}")
           for a in range(4)]
    # x1 window a holds wi in [wi0, wi0+nwi); fill from psA (wi 0..3) / psB (4..7)
    for a in range(4):
        wi0, nwi = X1_WIN[a]
        for ps, base, bkey in ((psA, 0, "blin0"), (psB, 4, "blin1")):
            lo = max(wi0, base)
            hi = min(wi0 + nwi, base + 4)
            if lo >= hi:
                continue
            evac(x1t[a], (lo - wi0) * 32, ps, (lo - base) * 32, (hi - lo) * 32, bkey,
                 slice(None))

    # ---- fused1 -> x3 windows ----
    x3t = [x3p.tile([X3_WIN[a][1] * 16, 5 * BCORE], BF16, tag=f"x3_{a}", name=f"x3_{a}")
           for a in range(4)]
    for g, (h0, nh) in enumerate(HG):
        for a in range(4):
            M = nh * 64
            ps = psmall.tile([M, BCORE], F32, tag="ps")
            nc.tensor.matmul(ps[:], W(("f1", g, a)), x1t[a][:], start=True, stop=True)
            for hi_ in range(nh):
                hh = h0 + hi_
                for b_ in range(4):
                    jb0, njb = X3_WIN[b_]
                    w_lo = max(4 * a, jb0)
                    w_hi = min(4 * a + 4, jb0 + njb)
                    if w_lo >= w_hi:
                        continue
                    evac(x3t[b_], (w_lo - jb0) * 16,
                         ps, hi_ * 64 + (w_lo - 4 * a) * 16, (w_hi - w_lo) * 16,
                         ("b1", g, a),
                         slice(hh * BCORE, (hh + 1) * BCORE))

    # ---- fused2 + fused3 per half ----
    a3t = []
    for t, (j0, nj, lo, hi) in enumerate(A3_WIN):
        at = a3p.tile([97, 25 * HALF], BF16, tag=f"a3_{t}", name=f"a3_{t}")
        a3t.append(at)
        nc.gpsimd.memset(at[96:97, :], 1.0)

    def a3_targets(w_lo, w_hi):
        res = []
        for t, o_lo, o_hi in A3_OWN:
            lo_ = max(w_lo, o_lo)
            hi_ = min(w_hi, o_hi)
            if lo_ < hi_:
                res.append((t, lo_, hi_))
        return res

    f2ev = [0]

    def f2_evac(ps, p0, Hh, w_lo, w_hi, bkey, hb):
        for t, lo_, hi_ in a3_targets(w_lo, w_hi):
            d0 = (lo_ - A3_WIN[t][0]) * 8
            pr0 = p0 + (lo_ - w_lo) * 8
            n = (hi_ - lo_) * 8
            fsl = slice(Hh * HALF, (Hh + 1) * HALF)
            for off, cnt in _pieces(pr0, d0, n):
                act_lrelu(a3t[t][d0 + off:d0 + off + cnt, fsl],
                          ps[pr0 + off:pr0 + off + cnt, :],
                          BV(bkey, pr0 + off, cnt))

    for half in range(1):
        hb = 0

        def xsl(i):
            return slice(i * BCORE + hb, i * BCORE + hb + HALF)

        for i in range(5):
            for a in range(4):
                ps = psmall.tile([128, HALF], F32, tag="ps", name=f"p12_{half}_{i}_{a}")
                nc.tensor.matmul(ps[:], WB(("f2r12", a)), x3t[a][:, xsl(i)],
                                 start=True, stop=True)
                for ri, r in enumerate((1, 2)):
                    f2_evac(ps, ri * 64, 5 * i + r, 8 * a, 8 * a + 8,
                            ("b2r12", a), hb)
                ps = psmall.tile([64, HALF], F32, tag="ps", name=f"p3_{half}_{i}_{a}")
                nc.tensor.matmul(ps[:], WB(("f2r3", a)), x3t[a][:, xsl(i)],
                                 start=True, stop=True)
                f2_evac(ps, 0, 5 * i + 3, 8 * a, 8 * a + 8, ("b2r3", a), hb)
                ps = psmall.tile([64, HALF], F32, tag="ps", name=f"p0_{half}_{i}_{a}")
                nc.tensor.matmul(ps[:], WB(("f2r0", a, 0)), x3t[a][:, xsl(i)],
                                 start=True, stop=(i == 0))
                if i > 0:
                    nc.tensor.matmul(ps[:], WB(("f2r0", a, -1)), x3t[a][:, xsl(i - 1)],
                                     start=False, stop=True)
                f2_evac(ps, 0, 5 * i, 8 * a, 8 * a + 8,
                        ("b2r0", a, "edge" if i == 0 else "mid"), hb)
                ps = psmall.tile([64, HALF], F32, tag="ps", name=f"p4_{half}_{i}_{a}")
                nc.tensor.matmul(ps[:], WB(("f2r4", a, 0)), x3t[a][:, xsl(i)],
                                 start=True, stop=(i == 4))
                if i < 4:
                    nc.tensor.matmul(ps[:], WB(("f2r4", a, 1)), x3t[a][:, xsl(i + 1)],
                                     start=False, stop=True)
                f2_evac(ps, 0, 5 * i + 4, 8 * a, 8 * a + 8,
                        ("b2r4", a, "edge" if i == 4 else "mid"), hb)

        # halo mirrors: t1 j 8..11 <- t0 rows 64..96; t2 j 16..19 <- t1 rows 64..96;
        # t3 j 20..27 <- t2 rows 32..96
        for dst, src_t, s0, d0, n in ((1, 0, 64, 0, 32), (2, 1, 64, 0, 32),
                                      (3, 2, 32, 0, 64)):
            for i5 in range(5):
                fsl = slice(i5 * 5 * HALF, (i5 + 1) * 5 * HALF)
                nc.sync.dma_start(a3t[dst][d0:d0 + n, fsl],
                                  a3t[src_t][s0:s0 + n, fsl])
        ps_ctx.close()
        pf3 = ctx.enter_context(tc.tile_pool(name="psB", bufs=4, space="PSUM"))

        # ---- fused3 ----
        stg_cnt = [0]
        for c in range(4):
            cb = c * CH
            for ip0 in range(0, 25, 2):
                np_ = min(2, 25 - ip0)
                stg = stgp.tile([128, 768 * np_], F32, tag="stg",
                                name=f"stg_{half}_{c}_{ip0}")
                stv = stg[:].rearrange("p (c4 hq jc s) -> p hq c4 jc s",
                                       c4=6, hq=2 * np_, jc=32, s=2)
                for ii in range(np_):
                    i = ip0 + ii
                    iclass = 1 if i == 0 else (2 if i == 24 else 0)
                    for t, (j0, nj, lo, hi) in enumerate(A3_WIN):
                        njc = hi - lo + 1
                        N = 4 * njc * 6
                        hN = N // 2

                        def lsl(ix):
                            return a3t[t][:, ix * HALF + cb: ix * HALF + cb + CH]

                        ps3 = pf3.tile([128, 264], F32, tag=f"f3_{t % 2}",
                                       name=f"ps3_{half}_{c}_{i}_{t}")
                        ps3 = ps3[:, 0:N]
                        mms = [(ps3[:, 0:N], WB(("f3", t, 0, iclass)), lsl(i))]
                        if i > 0:
                            mms.append((ps3[:, 0:hN], WB(("f3", t, -1))[:, 0:hN],
                                        lsl(i - 1)))
                        if i < 24:
                            mms.append((ps3[:, hN:N], WB(("f3", t, 1))[:, hN:N],
                                        lsl(i + 1)))
                        for k, (o_, w_, l_) in enumerate(mms):
                            nc.tensor.matmul(o_, l_, w_, start=(k == 0),
                                             stop=(k == len(mms) - 1),
                                             skip_group_check=True)
                        view = stv[:, 2 * ii:2 * ii + 2, :, lo:hi + 1, :]
                        k13 = stg_cnt[0] % 10
                        stg_cnt[0] += 1
                        if k13 < 5:
                            tmp = tmpp.tile([128, 264], F32, tag="f3tmp",
                                            name=f"tmp_{half}_{c}_{i}_{t}")
                            nc.vector.tensor_copy(tmp[:, :N], ps3[:])
                            for r_ in range(2):
                                vr = stv[:, 2 * ii + r_:2 * ii + r_ + 1, :,
                                         lo:hi + 1, :].squeeze(1)
                                nc.vector.scalar_tensor_tensor(
                                    vr, tmp[:, r_ * hN:(r_ + 1) * hN],
                                    0.01, tmp[:, r_ * hN:(r_ + 1) * hN],
                                    op0=OP.mult, op1=OP.max)
                        else:
                            nc.scalar.activation(view, ps3[:], AF.Lrelu, bias=0.0,
                                                 scale=1.0, alpha=0.01)
                bg = hb + cb
                dview = out_ap[bg:bg + CH, :, 2 * ip0:2 * ip0 + 2 * np_, :]
                sview = stg[:].rearrange("p (c h w) -> p c h w",
                                         c=6, h=2 * np_, w=64)
                nc.sync.dma_start(dview, sview)


def kernel(**inputs):
    inputs = {k: np.asarray(v) for k, v in inputs.items()}
    wp, bp, wb = _make_packs(inputs)
    wpack = wp.build()
    bpack = bp.build()
    import ml_dtypes
    wbpack = wb.build().astype(ml_dtypes.bfloat16)
    _build_program.wreg = wp.reg
    _build_program.breg = bp.reg
    _build_program.wbreg = wb.reg
    nc = _build_program(wpack.shape[1], bpack.shape[1], wbpack.shape[1])

    lat = np.ascontiguousarray(inputs["latent"].astype(np.float32))
    in_maps = [
        {"latent": lat[i * BCORE:(i + 1) * BCORE], "wpack": wpack,
         "bpack": bpack, "wbpack": wbpack}
        for i in range(NCORES)
    ]
    res = run_bass_kernel_spmd(nc, in_maps, core_ids=list(range(NCORES)))
    return np.concatenate([res.results[i]["out"] for i in range(NCORES)], axis=0)



# revision 13
# speedup vs baseline: 1.6719x; 1.6719x over previous
"""Trainium2 Bass kernel for nn_BetaVAEMark7Decoder (v2).

All six layers are matmuls on the TensorEngine; conv pairs are fused on the
host into banded composite blocks (up1*tc1, up2*tc2, up3*tc3). Data-parallel
over batch: 4096 rows split 512 per core.

v2 structural changes vs the 406us baseline:
- fused3 runs as stationary-reuse streams: per (batch-chunk, j-window) the
  a3 activation slice for input row i is loaded once and fires 1-2 merged
  matmuls into a rolling 2-slot-per-bank PSUM ring, relying on PSUM
  has_written semantics (accumulate where written, overwrite where not).
- j-windows (0,9),(4,13),(12,13),(20,12) with 8-wide ownership; window rows
  are permuted so owned rows sit at [0:64) making every fused2 evacuation a
  single full-width [64,512] instruction; halo rows filled by SBUF DMAs.
- biases folded into the matmuls via ones-rows (x3 and a3) so all evacs are
  single-pass lrelu, round-robined across Scalar and Vector engines.
- output staged in bf16 (c4-major, 4 h-rows per tile -> 512B descriptors),
  upcast to f32 on the host.
"""
import numpy as np
from contextlib import ExitStack

import concourse.bass as bass
import concourse.tile as tile
from concourse import bacc, mybir
from concourse.bass_utils import run_bass_kernel_spmd

F32 = mybir.dt.float32
F32R = mybir.dt.float32r
BF16 = mybir.dt.bfloat16
AF = mybir.ActivationFunctionType
OP = mybir.AluOpType

NCORES = 8
BCORE = 512

# fused3 (a3) j-windows over j=W2 in [0,32): (j0, nj); window t owns j in [8t, 8t+8)
F3_WIN = [(0, 9), (4, 13), (12, 13), (20, 12)]
# fused2 input (x3) windows over j=W1 in [0,16): (j0, nj); window a primary j in [4a, 4a+4)
X2_WIN = [(0, 5), (2, 7), (6, 7), (10, 6)]
# fused1 input (x1) windows over wi in [0,8)
X1_WIN = [(0, 3), (1, 4), (3, 4), (5, 3)]
HG = [(0, 2), (2, 2), (4, 1)]


def _x3_row(a, j):
    """Row base (of 16) for x2-col j in x3 window a: primary [0:64), halos after."""
    j0, nj = X2_WIN[a]
    p0 = 4 * a
    if p0 <= j < p0 + 4:
        return (j - p0) * 16
    if j < p0:
        return 64 + (j - j0) * 16
    return 64 + (p0 - j0) * 16 + (j - (p0 + 4)) * 16


def _x3_ones(a):
    return X2_WIN[a][1] * 16


def _a3_row(t, j):
    """Row base (of 8) for W2-col j in a3 window t: owned [0:64), halos after."""
    j0, nj = F3_WIN[t]
    p0 = 8 * t
    if p0 <= j < p0 + 8:
        return (j - p0) * 8
    if j < p0:
        return 64 + (j - j0) * 8
    return 64 + (p0 - j0) * 8 + (j - (p0 + 8)) * 8


def _a3_ones(t):
    return F3_WIN[t][1] * 8


# ---------------- host-side weight factorization ----------------
def _precompute(w):
    P = {}
    w_lin, b_lin = w["w_lin"], w["b_lin"]
    lhs_lin = np.zeros((7, 256), np.float32)
    c_lin = np.zeros(256, np.float32)
    for wi in range(8):
        for ci in range(32):
            lhs_lin[:, wi * 32 + ci] = w_lin[:, ci * 8 + wi]
            c_lin[wi * 32 + ci] = b_lin[ci * 8 + wi]
    P["lhs_lin"], P["c_lin"] = lhs_lin, c_lin

    w_up1, b_up1, w_tc1, b_tc1 = w["w_up1"], w["b_up1"], w["w_tc1"], w["b_tc1"]
    K1 = np.zeros((5, 2, 3, 32, 16), np.float32)
    for hh in range(5):
        for s in range(2):
            for dh in range(3):
                hp = hh + 1 - dh
                if not (0 <= hp < 5):
                    continue
                for dw in range(3):
                    t = s + 1 - dw
                    dj = int(np.floor(t / 2))
                    kw = t - 2 * dj
                    K1[hh, s, dj + 1] += np.einsum("ic,cd->id", w_up1[hp, kw], w_tc1[dh, dw])
    c1 = np.zeros((5, 16, 16), np.float32)
    for hh in range(5):
        for ww in range(16):
            acc = b_tc1.copy()
            for dh in range(3):
                if not (0 <= hh + 1 - dh < 5):
                    continue
                for dw in range(3):
                    if not (0 <= ww + 1 - dw < 16):
                        continue
                    acc = acc + b_up1 @ w_tc1[dh, dw]
            c1[hh, ww] = acc
    P["K1"], P["c1"] = K1, c1

    w_up2, b_up2, w_tc2, b_tc2 = w["w_up2"], w["b_up2"], w["w_tc2"], w["b_tc2"]
    K2 = np.zeros((5, 2, 3, 3, 16, 8), np.float32)
    for r in range(5):
        for s in range(2):
            for dh in range(3):
                u = r + 1 - dh
                di = int(np.floor(u / 5))
                kh = u - 5 * di
                for dw in range(3):
                    t = s + 1 - dw
                    dj = int(np.floor(t / 2))
                    kw = t - 2 * dj
                    K2[r, s, di + 1, dj + 1] += np.einsum("ic,cd->id", w_up2[kh, kw], w_tc2[dh, dw])
    P["K2"] = K2
    P["BB2"] = np.einsum("c,hwcd->hwd", b_up2, w_tc2)
    P["b_tc2"] = b_tc2

    w_up3, b_up3, w_tc3, b_tc3 = w["w_up3"], w["b_up3"], w["w_tc3"], w["b_tc3"]
    K3 = np.zeros((2, 2, 3, 3, 8, 6), np.float32)
    for r in range(2):
        for s in range(2):
            for dh in range(3):
                u = r + 1 - dh
                di = int(np.floor(u / 2))
                kh = u - 2 * di
                for dw in range(3):
                    t = s + 1 - dw
                    dj = int(np.floor(t / 2))
                    kw = t - 2 * dj
                    K3[r, s, di + 1, dj + 1] += np.einsum("ic,cd->id", w_up3[kh, kw], w_tc3[dh, dw])
    P["K3"] = K3
    P["BB3"] = np.einsum("c,hwcd->hwd", b_up3, w_tc3)
    P["b_tc3"] = b_tc3
    return P


def _fused1_blocks(P):
    K1 = P["K1"]
    blocks, biases = {}, {}
    for g, (h0, nh) in enumerate(HG):
        for a in range(4):
            wi0, nwi = X1_WIN[a]
            M = nh * 4 * 16
            B = np.zeros((nwi * 32, M), np.float32)
            bias = np.zeros(M, np.float32)
            for hi in range(nh):
                hh = h0 + hi
                for wl in range(4):
                    ww = 4 * a + wl
                    j, s = ww // 2, ww % 2
                    for c2 in range(16):
                        col = hi * 64 + wl * 16 + c2
                        bias[col] = P["c1"][hh, ww, c2]
                        for wi_l in range(nwi):
                            dj = (wi0 + wi_l) - j
                            if -1 <= dj <= 1:
                                B[wi_l * 32:(wi_l + 1) * 32, col] = K1[hh, s, dj + 1, :, c2]
            blocks[(g, a)] = B
            biases[(g, a)] = bias
    return blocks, biases


def _f2_col_bias(P, Hh, Ww, c3):
    acc = P["b_tc2"][c3]
    for dh in range(3):
        if not (0 <= Hh + 1 - dh < 25):
            continue
        for dw in range(3):
            if not (0 <= Ww + 1 - dw < 32):
                continue
            acc += P["BB2"][dh, dw, c3]
    return acc


def _fused2_blocks(P):
    """Blocks with x3 row permutation and bias rows at the ones-row position."""
    K2 = P["K2"]
    blocks = {}
    for a in range(4):
        j0, nj = X2_WIN[a]
        K = nj * 16 + 1
        ones = _x3_ones(a)

        def fill(B, colbase, r, di, bias_i=None):
            for wl in range(8):
                Ww = 8 * a + wl
                j, s = Ww // 2, Ww % 2
                for c3 in range(8):
                    col = colbase + wl * 8 + c3
                    for j2 in range(j0, j0 + nj):
                        dj = j2 - j
                        if -1 <= dj <= 1:
                            rb = _x3_row(a, j2)
                            B[rb:rb + 16, col] = K2[r, s, di + 1, dj + 1, :, c3]
                    if bias_i is not None:
                        B[ones, col] = _f2_col_bias(P, 5 * bias_i + r, Ww, c3)

        B = np.zeros((K, 128), np.float32)
        fill(B, 0, 1, 0, bias_i=1)
        fill(B, 64, 2, 0, bias_i=1)
        blocks[("r12", a)] = B
        for tag, bi in (("mid", 2), ("edge", 0)):
            B = np.zeros((K, 128), np.float32)
            fill(B, 0, 0, 0, bias_i=bi)
            fill(B, 64, 3, 0, bias_i=1)
            blocks[("m", tag, a)] = B
        B = np.zeros((K, 64), np.float32)
        fill(B, 0, 0, -1)
        blocks[("r0m1", a)] = B
        for tag, bi in (("mid", 2), ("edge", 4)):
            B = np.zeros((K, 64), np.float32)
            fill(B, 0, 4, 0, bias_i=bi)
            blocks[("r4", tag, a)] = B
        B = np.zeros((K, 64), np.float32)
        fill(B, 0, 4, 1)
        blocks[("r4p1", a)] = B
    return blocks


def _fused3_blocks(P):
    """Per t: cat [K,384] = [W(+1)r1 | W(0) | W(-1)r0], e0 [K,288], e24 [K,192].
    Slot col order r*96 + c4*16 + (jc-8t)*2 + s; a3 row permutation applied."""
    K3, BB3, b_tc3 = P["K3"], P["BB3"], P["b_tc3"]
    blocks = {}
    for t in range(4):
        j0, nj = F3_WIN[t]
        K = nj * 8 + 1
        ones = _a3_ones(t)

        def w_block(di, rsel, iclass=None):
            B = np.zeros((K, len(rsel) * 96), np.float32)
            for ri, r in enumerate(rsel):
                for c4 in range(6):
                    for jc in range(8 * t, 8 * t + 8):
                        for s in range(2):
                            col = ri * 96 + c4 * 16 + (jc - 8 * t) * 2 + s
                            for j2 in range(j0, j0 + nj):
                                dj = j2 - jc
                                if -1 <= dj <= 1:
                                    rb = _a3_row(t, j2)
                                    B[rb:rb + 8, col] = K3[r, s, di + 1, dj + 1, :, c4]
                            if iclass is not None and di == 0:
                                acc = b_tc3[c4]
                                for dh in range(3):
                                    u = r + 1 - dh
                                    di_ = int(np.floor(u / 2))
                                    ok = (iclass == 0) or (iclass == 1 and di_ >= 0) \
                                        or (iclass == 2 and di_ <= 0)
                                    if not ok:
                                        continue
                                    for dw in range(3):
                                        tt = s + 1 - dw
                                        dj_ = int(np.floor(tt / 2))
                                        if 0 <= jc + dj_ < 32:
                                            acc += BB3[dh, dw, c4]
                                B[ones, col] = acc
            return B

        w1r1 = w_block(1, [1])
        wm1r0 = w_block(-1, [0])
        blocks[("cat", t)] = np.concatenate([w1r1, w_block(0, [0, 1], 0), wm1r0], axis=1)
        blocks[("e0", t)] = np.concatenate([w_block(0, [0, 1], 1), wm1r0], axis=1)
        blocks[("e24", t)] = w_block(0, [0, 1], 2)
    return blocks


class _Pack:
    def __init__(self):
        self.cols = 0
        self.reg = {}
        self.items = []

    def add(self, key, arr):
        K, M = arr.shape
        self.reg[key] = (self.cols, K, M)
        self.items.append(arr)
        self.cols += M

    def build(self):
        out = np.zeros((128, self.cols), np.float32)
        c = 0
        for arr in self.items:
            K, M = arr.shape
            out[:K, c:c + M] = arr
            c += M
        return out


def _make_packs(inputs):
    P = _precompute(inputs)
    f1b, f1bias = _fused1_blocks(P)
    f2b = _fused2_blocks(P)
    f3b = _fused3_blocks(P)

    wp = _Pack()
    wp.add("lin0", P["lhs_lin"][:, 0:128])
    wp.add("lin1", P["lhs_lin"][:, 128:256])
    for g in range(3):
        for a in range(4):
            wp.add(("f1", g, a), f1b[(g, a)])

    wb = _Pack()
    for a in range(4):
        for key in [("r12", a), ("m", "mid", a), ("m", "edge", a), ("r0m1", a),
                    ("r4", "mid", a), ("r4", "edge", a), ("r4p1", a)]:
            wb.add(key, f2b[key])
    for t in range(4):
        for key in [("cat", t), ("e0", t), ("e24", t)]:
            wb.add(key, f3b[key])

    bp = _Pack()
    bp.add("blin0", P["c_lin"][0:128].reshape(-1, 1))
    bp.add("blin1", P["c_lin"][128:256].reshape(-1, 1))
    for g in range(3):
        for a in range(4):
            bp.add(("b1", g, a), f1bias[(g, a)].reshape(-1, 1))
    return wp, bp, wb


# ---------------- device program ----------------
_PROG = {}


def _lim(s):
    if s == 0:
        return 128
    if s == 64:
        return 64
    return 32


def _pieces(p0, d0, n):
    assert p0 % 32 == 0 and d0 % 32 == 0, (p0, d0, n)
    out = []
    off = 0
    while off < n:
        s1, s2 = (p0 + off) % 128, (d0 + off) % 128
        c = min(n - off, _lim(s1), _lim(s2))
        out.append((off, c))
        off += c
    return out


def _build_program(wcols, bcols, wbcols):
    key = (wcols, bcols, wbcols)
    if key in _PROG:
        return _PROG[key]
    nc = bacc.Bacc("TRN2", target_bir_lowering=False, debug=False, num_devices=NCORES)
    lat_ap = nc.dram_tensor("latent", [BCORE, 7], F32, kind="ExternalInput").ap()
    wp_ap = nc.dram_tensor("wpack", [128, wcols], F32, kind="ExternalInput").ap()
    bp_ap = nc.dram_tensor("bpack", [128, bcols], F32, kind="ExternalInput").ap()
    wb_ap = nc.dram_tensor("wbpack", [128, wbcols], BF16, kind="ExternalInput").ap()
    # h-major output (host transposes to NCHW): fully contiguous stg DMA
    out_ap = nc.dram_tensor("out", [BCORE, 50, 6, 64], BF16, kind="ExternalOutput").ap()
    with tile.TileContext(nc) as tc:
        with ExitStack() as ctx:
            _emit(ctx, tc, nc, lat_ap, wp_ap, bp_ap, wb_ap, out_ap,
                  _build_program.wreg, _build_program.breg, _build_program.wbreg)
    nc.compile()
    _PROG[key] = nc
    return nc


def _emit(ctx, tc, nc, lat_ap, wp_ap, bp_ap, wb_ap, out_ap, wreg, breg, wbreg):
    wcols = wp_ap.shape[1]
    bcols = bp_ap.shape[1]
    wbcols = wb_ap.shape[1]

    consts = ctx.enter_context(tc.tile_pool(name="consts", bufs=1))
    bounce = ctx.enter_context(tc.tile_pool(name="bounce", bufs=2))
    x1p = ctx.enter_context(tc.tile_pool(name="x1", bufs=1))
    x3p = ctx.enter_context(tc.tile_pool(name="x3", bufs=1))
    a3p = ctx.enter_context(tc.tile_pool(name="a3", bufs=1))
    stgp = ctx.enter_context(tc.tile_pool(name="stg", bufs=14))
    psp = ctx.enter_context(tc.tile_pool(name="ps", bufs=8, space="PSUM"))

    # ---- constants ----
    wp_r = consts.tile([128, wcols], F32R)
    for c0 in range(0, wcols, 512):
        n = min(512, wcols - c0)
        bt = bounce.tile([128, 512], F32, tag="bounce", name=f"bw{c0}")
        nc.sync.dma_start(bt[:, :n], wp_ap[:, c0:c0 + n])
        nc.vector.tensor_copy(wp_r[:, c0:c0 + n], bt[:, :n])
    wbt = consts.tile([128, wbcols], BF16)
    nc.sync.dma_start(wbt[:], wb_ap[:])
    bpt = consts.tile([128, bcols], F32)
    nc.sync.dma_start(bpt[:], bp_ap[:])
    lat_f = consts.tile([7, BCORE], F32)
    nc.sync.dma_start(lat_f[:], lat_ap[:].rearrange("b d -> d b"))
    lat_r = consts.tile([7, BCORE], F32R)
    nc.vector.tensor_copy(lat_r[:], lat_f[:])

    def W(key):
        o, K, M = wreg[key]
        return wp_r[:K, o:o + M]

    def WB(key, c0=None, c1=None):
        o, K, M = wbreg[key]
        if c0 is None:
            return wbt[:K, o:o + M]
        return wbt[:K, o + c0:o + c1]

    def BV(key, p0, n):
        o, K, M = breg[key]
        return bpt[p0:p0 + n, o:o + 1]

    # evac engines: ACT does lrelu via activation, DVE via scalar_tensor_tensor
    ev_ctr = [0]

    def ev_lrelu(dst, src):
        # DVE cannot read two PSUM operands in one instruction: its path is a
        # PSUM->SBUF copy + in-place all-SBUF bf16 lrelu (2x/4x DVE modes).
        if ev_ctr[0] % 2 == 0:
            nc.scalar.activation(dst, src, AF.Lrelu, bias=0.0, scale=1.0, alpha=0.01)
        else:
            nc.vector.tensor_copy(dst, src)
            nc.vector.scalar_tensor_tensor(dst, dst, 0.01, dst, op0=OP.mult, op1=OP.max)
        ev_ctr[0] += 1

    # ---- x3 / a3 tiles + ones rows ----
    x3t = [x3p.tile([X2_WIN[a][1] * 16 + 1, 5 * BCORE], BF16, tag=f"x3_{a}",
                    name=f"x3_{a}") for a in range(4)]
    a3t = [a3p.tile([F3_WIN[t][1] * 8 + 1, 25 * BCORE], BF16, tag=f"a3_{t}",
                    name=f"a3_{t}") for t in range(4)]
    # memset partition base must be 32-aligned; start below the ones row and
    # cover halo rows (overwritten later by the mirror DMAs before any read)
    for a in range(4):
        o = _x3_ones(a)
        nc.gpsimd.memset(x3t[a][o - o % 32:o + 1, :], 1.0)
    for t in range(4):
        o = _a3_ones(t)
        nc.gpsimd.memset(a3t[t][o - o % 32:o + 1, :], 1.0)

    # ---- lin ----
    psA = psp.tile([128, BCORE], F32, tag="ps")
    nc.tensor.matmul(psA[:], W("lin0"), lat_r[:], start=True, stop=True)
    psB = psp.tile([128, BCORE], F32, tag="ps")
    nc.tensor.matmul(psB[:], W("lin1"), lat_r[:], start=True, stop=True)

    x1t = [x1p.tile([X1_WIN[a][1] * 32, BCORE], F32R, tag=f"x1_{a}", name=f"x1_{a}")
           for a in range(4)]
    for a in range(4):
        wi0, nwi = X1_WIN[a]
        for ps, base, bkey in ((psA, 0, "blin0"), (psB, 4, "blin1")):
            lo = max(wi0, base)
            hi = min(wi0 + nwi, base + 4)
            if lo >= hi:
                continue
            d0 = (lo - wi0) * 32
            p0 = (lo - base) * 32
            n = (hi - lo) * 32
            for off, cnt in _pieces(p0, d0, n):
                nc.scalar.activation(x1t[a][d0 + off:d0 + off + cnt, :],
                                     ps[p0 + off:p0 + off + cnt, :],
                                     AF.Lrelu, bias=BV(bkey, p0 + off, cnt),
                                     scale=1.0, alpha=0.01)

    # ---- fused1 (a-outer so x3 mirrors can fire early) ----
    # x3 mirrors: (dst_a, d0, src_a, s0, n)
    X3_MIR = [(0, 64, 1, 0, 16), (1, 64, 0, 32, 32), (1, 96, 2, 0, 16),
              (2, 64, 1, 32, 32), (2, 96, 3, 0, 16), (3, 64, 2, 32, 32)]
    for a in range(4):
        for g, (h0, nh) in enumerate(HG):
            M = nh * 64
            ps = psp.tile([128, BCORE], F32, tag="ps", name=f"f1_{g}_{a}")
            nc.tensor.matmul(ps[0:M, :], W(("f1", g, a)), x1t[a][:],
                             start=True, stop=True)
            for hi_ in range(nh):
                hh = h0 + hi_
                # primary region of window a: rows [0:64) = W1 4a..4a+4
                nc.scalar.activation(
                    x3t[a][0:64, hh * BCORE:(hh + 1) * BCORE],
                    ps[hi_ * 64:hi_ * 64 + 64, :],
                    AF.Lrelu, bias=BV(("b1", g, a), hi_ * 64, 64),
                    scale=1.0, alpha=0.01)
        for dst_a, d0, src_a, s0, n in X3_MIR:
            if src_a == a:
                nc.sync.dma_start(x3t[dst_a][d0:d0 + n, :], x3t[a][s0:s0 + n, :])

    # ---- fused2 (a-outer so a3 mirrors can fire early) ----
    # a3 mirrors: (dst_t, d0, src_t, s0, n)
    A3_MIR = [(0, 64, 1, 0, 8), (1, 64, 0, 32, 32), (1, 96, 2, 0, 8),
              (2, 64, 1, 32, 32), (2, 96, 3, 0, 8), (3, 64, 2, 32, 32)]

    def xsl(a, i):
        K = X2_WIN[a][1] * 16 + 1
        return x3t[a][0:K, i * BCORE:(i + 1) * BCORE]

    for a in range(4):
        for i in range(5):
            ps1 = psp.tile([128, BCORE], F32, tag="ps", name=f"f2a_{a}_{i}")
            nc.tensor.matmul(ps1[:], WB(("r12", a)), xsl(a, i), start=True, stop=True)
            ps2 = psp.tile([128, BCORE], F32, tag="ps", name=f"f2b_{a}_{i}")
            mtag = "edge" if i == 0 else "mid"
            nc.tensor.matmul(ps2[:], WB(("m", mtag, a)), xsl(a, i),
                             start=True, stop=(i == 0), skip_group_check=True)
            if i > 0:
                nc.tensor.matmul(ps2[0:64, :], WB(("r0m1", a)), xsl(a, i - 1),
                                 start=False, stop=True, skip_group_check=True)
            ps3 = psp.tile([128, BCORE], F32, tag="ps", name=f"f2c_{a}_{i}")
            rtag = "edge" if i == 4 else "mid"
            nc.tensor.matmul(ps3[0:64, :], WB(("r4", rtag, a)), xsl(a, i),
                             start=True, stop=(i == 4), skip_group_check=True)
            if i < 4:
                nc.tensor.matmul(ps3[0:64, :], WB(("r4p1", a)), xsl(a, i + 1),
                                 start=False, stop=True, skip_group_check=True)
            # evacs: all into owned rows [0:64) of window a's a3... (owner = a)
            for ps, p0, r in ((ps1, 0, 1), (ps1, 64, 2), (ps2, 0, 0),
                              (ps2, 64, 3), (ps3, 0, 4)):
                H = 5 * i + r
                ev_lrelu(a3t[a][0:64, H * BCORE:(H + 1) * BCORE],
                         ps[p0:p0 + 64, :])
        for dst_t, d0, src_t, s0, n in A3_MIR:
            if src_t == a:
                nc.sync.dma_start(a3t[dst_t][d0:d0 + n, :], a3t[a][s0:s0 + n, :])

    # ---- fused3: merged-MM streams ----
    # bank k = [slot 2k | slot 2k+1], slot = 192 cols (r*96 + c4*16 + wloc).
    for c in range(4):
        cb = c * 128
        stg = [stgp.tile([128, 1536], BF16, tag="stg", name=f"stg_{c}_{k}")
               for k in range(13)]
        banks = {}
        for t in range(4):
            Kt = F3_WIN[t][1] * 8 + 1

            def st(i):
                return a3t[t][0:Kt, i * BCORE + cb:i * BCORE + cb + 128]

            for k in range(13):
                banks[(t, k)] = psp.tile([128, 512], F32, tag="ps",
                                         name=f"f3_{c}_{t}_{k}")

            def evac(k):
                ps = banks[(t, k)]
                n = 384 if k < 12 else 192
                hq = 4 if k < 12 else 2
                # stg cols (h, c4, w) h-major: PSUM order (slot,r,c4,w) is
                # h-major too, so dst collapses to 3D (p, h*c4 chunk, w)
                sv = stg[k][:].rearrange("p (hc w) -> p hc w", hc=24, w=64)
                ev_lrelu(sv[:, 0:hq * 6, 16 * t:16 * t + 16], ps[:, 0:n])
                if t == 3:
                    dview = out_ap[cb:cb + 128, 4 * k:4 * k + hq, :, :]
                    nc.sync.dma_start(dview, stg[k][:, 0:hq * 384])

            cat = ("cat", t)
            for i in range(25):
                k = i // 2
                if i == 0:
                    nc.tensor.matmul(banks[(t, 0)][:, 0:288], st(i), WB(("e0", t)),
                                     start=True, stop=False, skip_group_check=True)
                elif i % 2 == 1:
                    nc.tensor.matmul(banks[(t, k)][:, 96:384], st(i), WB(cat, 0, 288),
                                     start=False, stop=False, skip_group_check=True)
                    nc.tensor.matmul(banks[(t, k + 1)][:, 0:96], st(i),
                                     WB(cat, 288, 384), start=True, stop=False,
                                     skip_group_check=True)
                elif i < 24:
                    nc.tensor.matmul(banks[(t, k - 1)][:, 288:384], st(i),
                                     WB(cat, 0, 96), start=False, stop=True,
                                     skip_group_check=True)
                    nc.tensor.matmul(banks[(t, k)][:, 0:288], st(i), WB(cat, 96, 384),
                                     start=False, stop=False, skip_group_check=True)
                    evac(k - 1)
                else:
                    nc.tensor.matmul(banks[(t, 11)][:, 288:384], st(i),
                                     WB(cat, 0, 96), start=False, stop=True,
                                     skip_group_check=True)
                    nc.tensor.matmul(banks[(t, 12)][:, 0:192], st(i), WB(("e24", t)),
                                     start=False, stop=True, skip_group_check=True)
                    evac(11)
                    evac(12)


def kernel(**inputs):
    inputs = {k: np.asarray(v) for k, v in inputs.items()}
    wp, bp, wb = _make_packs(inputs)
    wpack = wp.build()
    bpack = bp.build()
    import ml_dtypes
    wbpack = wb.build().astype(ml_dtypes.bfloat16)
    _build_program.wreg = wp.reg
    _build_program.breg = bp.reg
    _build_program.wbreg = wb.reg
    nc = _build_program(wpack.shape[1], bpack.shape[1], wbpack.shape[1])

    lat = np.ascontiguousarray(inputs["latent"].astype(np.float32))
    in_maps = [
        {"latent": lat[i * BCORE:(i + 1) * BCORE], "wpack": wpack,
         "bpack": bpack, "wbpack": wbpack}
        for i in range(NCORES)
    ]
    res = run_bass_kernel_spmd(nc, in_maps, core_ids=list(range(NCORES)))
    out = np.concatenate(
        [np.asarray(res.results[i]["out"]).astype(np.float32) for i in range(NCORES)],
        axis=0)
    return np.ascontiguousarray(out.transpose(0, 2, 1, 3))  # [B,50,6,64]->[B,6,50,64]


# revision 18
# speedup vs baseline: 1.9536x; 1.1685x over previous
"""Trainium2 Bass kernel for nn_BetaVAEMark7Decoder (v2).

All six layers are matmuls on the TensorEngine; conv pairs are fused on the
host into banded composite blocks (up1*tc1, up2*tc2, up3*tc3). Data-parallel
over batch: 4096 rows split 512 per core.

v2 structural changes vs the 406us baseline:
- fused3 runs as stationary-reuse streams: per (batch-chunk, j-window) the
  a3 activation slice for input row i is loaded once and fires 1-2 merged
  matmuls into a rolling 2-slot-per-bank PSUM ring, relying on PSUM
  has_written semantics (accumulate where written, overwrite where not).
- j-windows (0,9),(4,13),(12,13),(20,12) with 8-wide ownership; window rows
  are permuted so owned rows sit at [0:64) making every fused2 evacuation a
  single full-width [64,512] instruction; halo rows filled by SBUF DMAs.
- biases folded into the matmuls via ones-rows (x3 and a3) so all evacs are
  single-pass lrelu, round-robined across Scalar and Vector engines.
- output staged in bf16 (c4-major, 4 h-rows per tile -> 512B descriptors),
  upcast to f32 on the host.
"""
import numpy as np
from contextlib import ExitStack

import concourse.bass as bass
import concourse.tile as tile
from concourse import bacc, mybir
from concourse.bass_utils import run_bass_kernel_spmd

F32 = mybir.dt.float32
F32R = mybir.dt.float32r
BF16 = mybir.dt.bfloat16
AF = mybir.ActivationFunctionType
OP = mybir.AluOpType

NCORES = 8
BCORE = 512

# fused3 (a3) j-windows over j=W2 in [0,32): (j0, nj); window t owns j in [8t, 8t+8)
F3_WIN = [(0, 9), (4, 13), (12, 13), (20, 12)]
# fused2 input (x3) windows over j=W1 in [0,16): (j0, nj); window a primary j in [4a, 4a+4)
X2_WIN = [(0, 5), (2, 7), (6, 7), (10, 6)]
# fused1 input (x1) windows over wi in [0,8)
X1_WIN = [(0, 3), (1, 4), (3, 4), (5, 3)]
HG = [(0, 2), (2, 2), (4, 1)]


def _x3_row(a, j):
    """Row base (of 16) for x2-col j in x3 window a: primary [0:64), halos after."""
    j0, nj = X2_WIN[a]
    p0 = 4 * a
    if p0 <= j < p0 + 4:
        return (j - p0) * 16
    if j < p0:
        return 64 + (j - j0) * 16
    return 64 + (p0 - j0) * 16 + (j - (p0 + 4)) * 16


def _x3_ones(a):
    return X2_WIN[a][1] * 16


def _a3_row(t, j):
    """Row base (of 8) for W2-col j in a3 window t: owned [0:64), halos after."""
    j0, nj = F3_WIN[t]
    p0 = 8 * t
    if p0 <= j < p0 + 8:
        return (j - p0) * 8
    if j < p0:
        return 64 + (j - j0) * 8
    return 64 + (p0 - j0) * 8 + (j - (p0 + 8)) * 8


def _a3_ones(t):
    return F3_WIN[t][1] * 8


# ---------------- host-side weight factorization ----------------
def _precompute(w):
    P = {}
    w_lin, b_lin = w["w_lin"], w["b_lin"]
    lhs_lin = np.zeros((7, 256), np.float32)
    c_lin = np.zeros(256, np.float32)
    for wi in range(8):
        for ci in range(32):
            lhs_lin[:, wi * 32 + ci] = w_lin[:, ci * 8 + wi]
            c_lin[wi * 32 + ci] = b_lin[ci * 8 + wi]
    P["lhs_lin"], P["c_lin"] = lhs_lin, c_lin

    w_up1, b_up1, w_tc1, b_tc1 = w["w_up1"], w["b_up1"], w["w_tc1"], w["b_tc1"]
    K1 = np.zeros((5, 2, 3, 32, 16), np.float32)
    for hh in range(5):
        for s in range(2):
            for dh in range(3):
                hp = hh + 1 - dh
                if not (0 <= hp < 5):
                    continue
                for dw in range(3):
                    t = s + 1 - dw
                    dj = int(np.floor(t / 2))
                    kw = t - 2 * dj
                    K1[hh, s, dj + 1] += np.einsum("ic,cd->id", w_up1[hp, kw], w_tc1[dh, dw])
    c1 = np.zeros((5, 16, 16), np.float32)
    for hh in range(5):
        for ww in range(16):
            acc = b_tc1.copy()
            for dh in range(3):
                if not (0 <= hh + 1 - dh < 5):
                    continue
                for dw in range(3):
                    if not (0 <= ww + 1 - dw < 16):
                        continue
                    acc = acc + b_up1 @ w_tc1[dh, dw]
            c1[hh, ww] = acc
    P["K1"], P["c1"] = K1, c1

    w_up2, b_up2, w_tc2, b_tc2 = w["w_up2"], w["b_up2"], w["w_tc2"], w["b_tc2"]
    K2 = np.zeros((5, 2, 3, 3, 16, 8), np.float32)
    for r in range(5):
        for s in range(2):
            for dh in range(3):
                u = r + 1 - dh
                di = int(np.floor(u / 5))
                kh = u - 5 * di
                for dw in range(3):
                    t = s + 1 - dw
                    dj = int(np.floor(t / 2))
                    kw = t - 2 * dj
                    K2[r, s, di + 1, dj + 1] += np.einsum("ic,cd->id", w_up2[kh, kw], w_tc2[dh, dw])
    P["K2"] = K2
    P["BB2"] = np.einsum("c,hwcd->hwd", b_up2, w_tc2)
    P["b_tc2"] = b_tc2

    w_up3, b_up3, w_tc3, b_tc3 = w["w_up3"], w["b_up3"], w["w_tc3"], w["b_tc3"]
    K3 = np.zeros((2, 2, 3, 3, 8, 6), np.float32)
    for r in range(2):
        for s in range(2):
            for dh in range(3):
                u = r + 1 - dh
                di = int(np.floor(u / 2))
                kh = u - 2 * di
                for dw in range(3):
                    t = s + 1 - dw
                    dj = int(np.floor(t / 2))
                    kw = t - 2 * dj
                    K3[r, s, di + 1, dj + 1] += np.einsum("ic,cd->id", w_up3[kh, kw], w_tc3[dh, dw])
    P["K3"] = K3
    P["BB3"] = np.einsum("c,hwcd->hwd", b_up3, w_tc3)
    P["b_tc3"] = b_tc3
    return P


def _fused1_blocks(P):
    K1 = P["K1"]
    blocks, biases = {}, {}
    for g, (h0, nh) in enumerate(HG):
        for a in range(4):
            wi0, nwi = X1_WIN[a]
            M = nh * 4 * 16
            B = np.zeros((nwi * 32, M), np.float32)
            bias = np.zeros(M, np.float32)
            for hi in range(nh):
                hh = h0 + hi
                for wl in range(4):
                    ww = 4 * a + wl
                    j, s = ww // 2, ww % 2
                    for c2 in range(16):
                        col = hi * 64 + wl * 16 + c2
                        bias[col] = P["c1"][hh, ww, c2]
                        for wi_l in range(nwi):
                            dj = (wi0 + wi_l) - j
                            if -1 <= dj <= 1:
                                B[wi_l * 32:(wi_l + 1) * 32, col] = K1[hh, s, dj + 1, :, c2]
            blocks[(g, a)] = B
            biases[(g, a)] = bias
    return blocks, biases


def _f2_col_bias(P, Hh, Ww, c3):
    acc = P["b_tc2"][c3]
    for dh in range(3):
        if not (0 <= Hh + 1 - dh < 25):
            continue
        for dw in range(3):
            if not (0 <= Ww + 1 - dw < 32):
                continue
            acc += P["BB2"][dh, dw, c3]
    return acc


def _fused2_blocks(P):
    """Blocks with x3 row permutation and bias rows at the ones-row position."""
    K2 = P["K2"]
    blocks = {}
    for a in range(4):
        j0, nj = X2_WIN[a]
        K = nj * 16 + 1
        ones = _x3_ones(a)

        def fill(B, colbase, r, di, bias_i=None):
            for wl in range(8):
                Ww = 8 * a + wl
                j, s = Ww // 2, Ww % 2
                for c3 in range(8):
                    col = colbase + wl * 8 + c3
                    for j2 in range(j0, j0 + nj):
                        dj = j2 - j
                        if -1 <= dj <= 1:
                            rb = _x3_row(a, j2)
                            B[rb:rb + 16, col] = K2[r, s, di + 1, dj + 1, :, c3]
                    if bias_i is not None:
                        B[ones, col] = _f2_col_bias(P, 5 * bias_i + r, Ww, c3)

        B = np.zeros((K, 128), np.float32)
        fill(B, 0, 1, 0, bias_i=1)
        fill(B, 64, 2, 0, bias_i=1)
        blocks[("r12", a)] = B
        for tag, bi in (("mid", 2), ("edge", 0)):
            B = np.zeros((K, 128), np.float32)
            fill(B, 0, 0, 0, bias_i=bi)
            fill(B, 64, 3, 0, bias_i=1)
            blocks[("m", tag, a)] = B
        B = np.zeros((K, 64), np.float32)
        fill(B, 0, 0, -1)
        blocks[("r0m1", a)] = B
        for tag, bi in (("mid", 2), ("edge", 4)):
            B = np.zeros((K, 64), np.float32)
            fill(B, 0, 4, 0, bias_i=bi)
            blocks[("r4", tag, a)] = B
        B = np.zeros((K, 64), np.float32)
        fill(B, 0, 4, 1)
        blocks[("r4p1", a)] = B
    return blocks


def _fused3_blocks(P):
    """Per t: cat [K,384] = [W(+1)r1 | W(0) | W(-1)r0], e0 [K,288], e24 [K,192].
    Slot col order r*96 + c4*16 + (jc-8t)*2 + s; a3 row permutation applied."""
    K3, BB3, b_tc3 = P["K3"], P["BB3"], P["b_tc3"]
    blocks = {}
    for t in range(4):
        j0, nj = F3_WIN[t]
        K = nj * 8 + 1
        ones = _a3_ones(t)

        def w_block(di, rsel, iclass=None):
            B = np.zeros((K, len(rsel) * 96), np.float32)
            for ri, r in enumerate(rsel):
                for c4 in range(6):
                    for jc in range(8 * t, 8 * t + 8):
                        for s in range(2):
                            col = ri * 96 + c4 * 16 + (jc - 8 * t) * 2 + s
                            for j2 in range(j0, j0 + nj):
                                dj = j2 - jc
                                if -1 <= dj <= 1:
                                    rb = _a3_row(t, j2)
                                    B[rb:rb + 8, col] = K3[r, s, di + 1, dj + 1, :, c4]
                            if iclass is not None and di == 0:
                                acc = b_tc3[c4]
                                for dh in range(3):
                                    u = r + 1 - dh
                                    di_ = int(np.floor(u / 2))
                                    ok = (iclass == 0) or (iclass == 1 and di_ >= 0) \
                                        or (iclass == 2 and di_ <= 0)
                                    if not ok:
                                        continue
                                    for dw in range(3):
                                        tt = s + 1 - dw
                                        dj_ = int(np.floor(tt / 2))
                                        if 0 <= jc + dj_ < 32:
                                            acc += BB3[dh, dw, c4]
                                B[ones, col] = acc
            return B

        w1r1 = w_block(1, [1])
        wm1r0 = w_block(-1, [0])
        blocks[("cat", t)] = np.concatenate([w1r1, w_block(0, [0, 1], 0), wm1r0], axis=1)
        blocks[("e0", t)] = np.concatenate([w_block(0, [0, 1], 1), wm1r0], axis=1)
        blocks[("e24", t)] = w_block(0, [0, 1], 2)
    return blocks


class _Pack:
    def __init__(self):
        self.cols = 0
        self.reg = {}
        self.items = []

    def add(self, key, arr):
        K, M = arr.shape
        self.reg[key] = (self.cols, K, M)
        self.items.append(arr)
        self.cols += M

    def build(self):
        out = np.zeros((128, self.cols), np.float32)
        c = 0
        for arr in self.items:
            K, M = arr.shape
            out[:K, c:c + M] = arr
            c += M
        return out


def _make_packs(inputs):
    P = _precompute(inputs)
    f1b, f1bias = _fused1_blocks(P)
    f2b = _fused2_blocks(P)
    f3b = _fused3_blocks(P)

    wp = _Pack()
    wp.add("lin0", P["lhs_lin"][:, 0:128])
    wp.add("lin1", P["lhs_lin"][:, 128:256])
    for g in range(3):
        for a in range(4):
            wp.add(("f1", g, a), f1b[(g, a)])

    wb = _Pack()
    for a in range(4):
        for key in [("r12", a), ("m", "mid", a), ("m", "edge", a), ("r0m1", a),
                    ("r4", "mid", a), ("r4", "edge", a), ("r4p1", a)]:
            wb.add(key, f2b[key])
    for t in range(4):
        for key in [("cat", t), ("e0", t), ("e24", t)]:
            wb.add(key, f3b[key])

    bp = _Pack()
    bp.add("blin0", P["c_lin"][0:128].reshape(-1, 1))
    bp.add("blin1", P["c_lin"][128:256].reshape(-1, 1))
    for g in range(3):
        for a in range(4):
            bp.add(("b1", g, a), f1bias[(g, a)].reshape(-1, 1))
    return wp, bp, wb


# ---------------- device program ----------------
_PROG = {}


def _lim(s):
    if s == 0:
        return 128
    if s == 64:
        return 64
    return 32


def _pieces(p0, d0, n):
    assert p0 % 32 == 0 and d0 % 32 == 0, (p0, d0, n)
    out = []
    off = 0
    while off < n:
        s1, s2 = (p0 + off) % 128, (d0 + off) % 128
        c = min(n - off, _lim(s1), _lim(s2))
        out.append((off, c))
        off += c
    return out


def _build_program(wcols, bcols, wbcols):
    key = (wcols, bcols, wbcols)
    if key in _PROG:
        return _PROG[key]
    nc = bacc.Bacc("TRN2", target_bir_lowering=False, debug=False, num_devices=NCORES)
    lat_ap = nc.dram_tensor("latent", [BCORE, 7], F32, kind="ExternalInput").ap()
    wp_ap = nc.dram_tensor("wpack", [128, wcols], F32, kind="ExternalInput").ap()
    bp_ap = nc.dram_tensor("bpack", [128, bcols], F32, kind="ExternalInput").ap()
    wb_ap = nc.dram_tensor("wbpack", [128, wbcols], BF16, kind="ExternalInput").ap()
    # h-major output (host transposes to NCHW): fully contiguous stg DMA
    out_ap = nc.dram_tensor("out", [BCORE, 50, 6, 64], BF16, kind="ExternalOutput").ap()
    ones_ap = nc.dram_tensor("ones", [1, 25 * BCORE], BF16, kind="ExternalInput").ap()
    with tile.TileContext(nc) as tc:
        with ExitStack() as ctx:
            _emit(ctx, tc, nc, lat_ap, wp_ap, bp_ap, wb_ap, out_ap, ones_ap,
                  _build_program.wreg, _build_program.breg, _build_program.wbreg)
    _dedup_ldweights(nc)
    nc.compile()
    _PROG[key] = nc
    return nc


def _dedup_ldweights(nc):
    """Drop InstLdweights whose stationary AP matches the previous load on the
    PE queue (the PE array keeps the stationary across matmuls)."""
    from concourse import mybir
    removed = 0
    for fn in nc.m.functions:
        for blk in fn.blocks:
            insts = list(blk.instructions)
            keep = []
            prev_sig = None
            for ins in insts:
                tn = type(ins).__name__
                if tn == "InstLdweights":
                    sig = (str(ins.ins[0]), str(getattr(ins, "perf_mode", None)),
                           str(getattr(ins, "is_transpose", None)))
                    if sig == prev_sig and not ins.has_wait() and not ins.has_update():
                        removed += 1
                        continue
                    prev_sig = sig
                elif tn == "InstMatmult":
                    pass  # streaming doesn't clobber the loaded stationary
                elif getattr(ins, "engine", None) == mybir.EngineType.PE \
                        and tn not in ("InstEventSemaphore",):
                    prev_sig = None
                keep.append(ins)
            if len(keep) != len(insts):
                blk.instructions = keep
    return removed


def _emit(ctx, tc, nc, lat_ap, wp_ap, bp_ap, wb_ap, out_ap, ones_ap, wreg, breg, wbreg):
    wcols = wp_ap.shape[1]
    bcols = bp_ap.shape[1]
    wbcols = wb_ap.shape[1]

    consts = ctx.enter_context(tc.tile_pool(name="consts", bufs=1))
    bounce = ctx.enter_context(tc.tile_pool(name="bounce", bufs=2))
    x1p = ctx.enter_context(tc.tile_pool(name="x1", bufs=1))
    x3p = ctx.enter_context(tc.tile_pool(name="x3", bufs=1))
    a3p = ctx.enter_context(tc.tile_pool(name="a3", bufs=1))
    stgp = ctx.enter_context(tc.tile_pool(name="stg", bufs=14))
    psp = ctx.enter_context(tc.tile_pool(name="ps", bufs=8, space="PSUM"))

    # ---- constants ----
    wp_r = consts.tile([128, wcols], F32R)
    for c0 in range(0, wcols, 512):
        n = min(512, wcols - c0)
        bt = bounce.tile([128, 512], F32, tag="bounce", name=f"bw{c0}")
        nc.sync.dma_start(bt[:, :n], wp_ap[:, c0:c0 + n])
        nc.vector.tensor_copy(wp_r[:, c0:c0 + n], bt[:, :n])
    wbt = consts.tile([128, wbcols], BF16)
    nc.sync.dma_start(wbt[:], wb_ap[:])
    bpt = consts.tile([128, bcols], F32)
    nc.sync.dma_start(bpt[:], bp_ap[:])
    lat_f = consts.tile([7, BCORE], F32)
    nc.sync.dma_start(lat_f[:], lat_ap[:].rearrange("b d -> d b"))
    lat_r = consts.tile([7, BCORE], F32R)
    nc.vector.tensor_copy(lat_r[:], lat_f[:])

    def W(key):
        o, K, M = wreg[key]
        return wp_r[:K, o:o + M]

    def WB(key, c0=None, c1=None):
        o, K, M = wbreg[key]
        if c0 is None:
            return wbt[:K, o:o + M]
        return wbt[:K, o + c0:o + c1]

    def BV(key, p0, n):
        o, K, M = breg[key]
        return bpt[p0:p0 + n, o:o + 1]

    # evac engines: ACT does lrelu via activation, DVE via scalar_tensor_tensor
    ev_ctr = [0]

    def ev_lrelu(dst, src):
        # DVE cannot read two PSUM operands in one instruction: its path is a
        # PSUM->SBUF copy + in-place all-SBUF bf16 lrelu (2x/4x DVE modes).
        # ACT's single pass is cheaper, so it takes 3 of every 5 evacs.
        if ev_ctr[0] % 5 in (0, 2, 4):
            nc.scalar.activation(dst, src, AF.Lrelu, bias=0.0, scale=1.0, alpha=0.01)
        else:
            nc.vector.tensor_copy(dst, src)
            nc.vector.scalar_tensor_tensor(dst, dst, 0.01, dst, op0=OP.mult, op1=OP.max)
        ev_ctr[0] += 1

    # ---- x3 / a3 tiles + ones rows ----
    x3t = [x3p.tile([X2_WIN[a][1] * 16 + 1, 5 * BCORE], BF16, tag=f"x3_{a}",
                    name=f"x3_{a}") for a in range(4)]
    a3t = [a3p.tile([F3_WIN[t][1] * 8 + 1, 25 * BCORE], BF16, tag=f"a3_{t}",
                    name=f"a3_{t}") for t in range(4)]
    # ones rows via DMA from a DRAM constant (gpsimd memset is ~10us per row)
    for a in range(4):
        o = _x3_ones(a)
        nc.sync.dma_start(x3t[a][o:o + 1, :], ones_ap[0:1, 0:5 * BCORE])
    for t in range(4):
        o = _a3_ones(t)
        nc.sync.dma_start(a3t[t][o:o + 1, :], ones_ap[0:1, :])

    # ---- lin ----
    psA = psp.tile([128, BCORE], F32, tag="ps")
    nc.tensor.matmul(psA[:], W("lin0"), lat_r[:], start=True, stop=True)
    psB = psp.tile([128, BCORE], F32, tag="ps")
    nc.tensor.matmul(psB[:], W("lin1"), lat_r[:], start=True, stop=True)

    x1t = [x1p.tile([X1_WIN[a][1] * 32, BCORE], F32R, tag=f"x1_{a}", name=f"x1_{a}")
           for a in range(4)]
    for a in range(4):
        wi0, nwi = X1_WIN[a]
        for ps, base, bkey in ((psA, 0, "blin0"), (psB, 4, "blin1")):
            lo = max(wi0, base)
            hi = min(wi0 + nwi, base + 4)
            if lo >= hi:
                continue
            d0 = (lo - wi0) * 32
            p0 = (lo - base) * 32
            n = (hi - lo) * 32
            for off, cnt in _pieces(p0, d0, n):
                nc.scalar.activation(x1t[a][d0 + off:d0 + off + cnt, :],
                                     ps[p0 + off:p0 + off + cnt, :],
                                     AF.Lrelu, bias=BV(bkey, p0 + off, cnt),
                                     scale=1.0, alpha=0.01)

    # ---- fused1 (a-outer so x3 mirrors can fire early) ----
    # x3 mirrors: (dst_a, d0, src_a, s0, n)
    X3_MIR = [(0, 64, 1, 0, 16), (1, 64, 0, 32, 32), (1, 96, 2, 0, 16),
              (2, 64, 1, 32, 32), (2, 96, 3, 0, 16), (3, 64, 2, 32, 32)]
    for a in range(4):
        for g, (h0, nh) in enumerate(HG):
            M = nh * 64
            ps = psp.tile([128, BCORE], F32, tag="ps", name=f"f1_{g}_{a}")
            nc.tensor.matmul(ps[0:M, :], W(("f1", g, a)), x1t[a][:],
                             start=True, stop=True)
            for hi_ in range(nh):
                hh = h0 + hi_
                # primary region of window a: rows [0:64) = W1 4a..4a+4
                nc.scalar.activation(
                    x3t[a][0:64, hh * BCORE:(hh + 1) * BCORE],
                    ps[hi_ * 64:hi_ * 64 + 64, :],
                    AF.Lrelu, bias=BV(("b1", g, a), hi_ * 64, 64),
                    scale=1.0, alpha=0.01)
        for dst_a, d0, src_a, s0, n in X3_MIR:
            if src_a == a:
                nc.sync.dma_start(x3t[dst_a][d0:d0 + n, :], x3t[a][s0:s0 + n, :])

    # ---- fused2 (a-outer so a3 mirrors can fire early) ----
    # a3 mirrors: (dst_t, d0, src_t, s0, n)
    A3_MIR = [(0, 64, 1, 0, 8), (1, 64, 0, 32, 32), (1, 96, 2, 0, 8),
              (2, 64, 1, 32, 32), (2, 96, 3, 0, 8), (3, 64, 2, 32, 32)]

    def xsl(a, i):
        K = X2_WIN[a][1] * 16 + 1
        return x3t[a][0:K, i * BCORE:(i + 1) * BCORE]

    for a in range(4):
        for i in range(5):
            ps1 = psp.tile([128, BCORE], F32, tag="ps", name=f"f2a_{a}_{i}")
            nc.tensor.matmul(ps1[:], WB(("r12", a)), xsl(a, i), start=True, stop=True)
            ps2 = psp.tile([128, BCORE], F32, tag="ps", name=f"f2b_{a}_{i}")
            mtag = "edge" if i == 0 else "mid"
            nc.tensor.matmul(ps2[:], WB(("m", mtag, a)), xsl(a, i),
                             start=True, stop=(i == 0), skip_group_check=True)
            if i > 0:
                nc.tensor.matmul(ps2[0:64, :], WB(("r0m1", a)), xsl(a, i - 1),
                                 start=False, stop=True, skip_group_check=True)
            ps3 = psp.tile([128, BCORE], F32, tag="ps", name=f"f2c_{a}_{i}")
            rtag = "edge" if i == 4 else "mid"
            nc.tensor.matmul(ps3[0:64, :], WB(("r4", rtag, a)), xsl(a, i),
                             start=True, stop=(i == 4), skip_group_check=True)
            if i < 4:
                nc.tensor.matmul(ps3[0:64, :], WB(("r4p1", a)), xsl(a, i + 1),
                                 start=False, stop=True, skip_group_check=True)
            # evacs: all into owned rows [0:64) of window a's a3... (owner = a)
            for ps, p0, r in ((ps1, 0, 1), (ps1, 64, 2), (ps2, 0, 0),
                              (ps2, 64, 3), (ps3, 0, 4)):
                H = 5 * i + r
                ev_lrelu(a3t[a][0:64, H * BCORE:(H + 1) * BCORE],
                         ps[p0:p0 + 64, :])
        for dst_t, d0, src_t, s0, n in A3_MIR:
            if src_t == a:
                nc.sync.dma_start(a3t[dst_t][d0:d0 + n, :], a3t[a][s0:s0 + n, :])

    # ---- fused3: merged-MM streams ----
    # bank k = [slot 2k | slot 2k+1], slot = 192 cols (r*96 + c4*16 + wloc).
    for c in range(4):
        cb = c * 128
        stg = [stgp.tile([128, 1536], BF16, tag="stg", name=f"stg_{c}_{k}")
               for k in range(13)]
        banks = {}
        for t in range(4):
            Kt = F3_WIN[t][1] * 8 + 1

            def st(i):
                return a3t[t][0:Kt, i * BCORE + cb:i * BCORE + cb + 128]

            for k in range(13):
                banks[(t, k)] = psp.tile([128, 512], F32, tag="ps",
                                         name=f"f3_{c}_{t}_{k}")

            def evac(k):
                ps = banks[(t, k)]
                n = 384 if k < 12 else 192
                hq = 4 if k < 12 else 2
                # stg cols (h, c4, w) h-major: PSUM order (slot,r,c4,w) is
                # h-major too, so dst collapses to 3D (p, h*c4 chunk, w)
                sv = stg[k][:].rearrange("p (hc w) -> p hc w", hc=24, w=64)
                ev_lrelu(sv[:, 0:hq * 6, 16 * t:16 * t + 16], ps[:, 0:n])
                if t == 3:
                    dview = out_ap[cb:cb + 128, 4 * k:4 * k + hq, :, :]
                    nc.sync.dma_start(dview, stg[k][:, 0:hq * 384])

            cat = ("cat", t)
            for i in range(25):
                k = i // 2
                if i == 0:
                    nc.tensor.matmul(banks[(t, 0)][:, 0:288], st(i), WB(("e0", t)),
                                     start=True, stop=False, skip_group_check=True)
                elif i % 2 == 1:
                    nc.tensor.matmul(banks[(t, k)][:, 96:384], st(i), WB(cat, 0, 288),
                                     start=False, stop=False, skip_group_check=True)
                    nc.tensor.matmul(banks[(t, k + 1)][:, 0:96], st(i),
                                     WB(cat, 288, 384), start=True, stop=False,
                                     skip_group_check=True)
                elif i < 24:
                    nc.tensor.matmul(banks[(t, k - 1)][:, 288:384], st(i),
                                     WB(cat, 0, 96), start=False, stop=True,
                                     skip_group_check=True)
                    nc.tensor.matmul(banks[(t, k)][:, 0:288], st(i), WB(cat, 96, 384),
                                     start=False, stop=False, skip_group_check=True)
                    evac(k - 1)
                else:
                    nc.tensor.matmul(banks[(t, 11)][:, 288:384], st(i),
                                     WB(cat, 0, 96), start=False, stop=True,
                                     skip_group_check=True)
                    nc.tensor.matmul(banks[(t, 12)][:, 0:192], st(i), WB(("e24", t)),
                                     start=False, stop=True, skip_group_check=True)
                    evac(11)
                    evac(12)


def kernel(**inputs):
    inputs = {k: np.asarray(v) for k, v in inputs.items()}
    wp, bp, wb = _make_packs(inputs)
    wpack = wp.build()
    bpack = bp.build()
    import ml_dtypes
    wbpack = wb.build().astype(ml_dtypes.bfloat16)
    _build_program.wreg = wp.reg
    _build_program.breg = bp.reg
    _build_program.wbreg = wb.reg
    nc = _build_program(wpack.shape[1], bpack.shape[1], wbpack.shape[1])

    lat = np.ascontiguousarray(inputs["latent"].astype(np.float32))
    ones = np.ones((1, 25 * BCORE), ml_dtypes.bfloat16)
    in_maps = [
        {"latent": lat[i * BCORE:(i + 1) * BCORE], "wpack": wpack,
         "bpack": bpack, "wbpack": wbpack, "ones": ones}
        for i in range(NCORES)
    ]
    res = run_bass_kernel_spmd(nc, in_maps, core_ids=list(range(NCORES)))
    out = np.concatenate(
        [np.asarray(res.results[i]["out"]).astype(np.float32) for i in range(NCORES)],
        axis=0)
    return np.ascontiguousarray(out.transpose(0, 2, 1, 3))  # [B,50,6,64]->[B,6,50,64]
